# revision 19
# baseline (speedup 1.0000x reference)
"""LIF spike kernel for Trainium2 (Bass/Tile), data-parallel over 8 NeuronCores.

Problem: x [32, 8, 128, 32, 32] fp32 -> spikes [32, 8, 128, 32, 32] fp32
    mem_t = mem_{t-1} * 0.25 + x_t ; spike = (mem >= 0.5) ; mem *= (1 - spike)

Sharding: batch dim (32) split 4-per-core across 8 cores; no cross-core comm.

Per-core device program, variant "direct" (default), measured-driven design:
  - loads: x host-sharded to c-major [4, 128, 8, 1024]; per batch FOUR 1 MiB
    dma_start jobs ([128, 2, 1024] slices, 8 KiB per-partition descriptors).
    Measured: chunked jobs sustain ~1.1 TB/s vs ~0.6 TB/s for one 4 MiB job,
    so the 16.8 MB read floor is ~15.5 us.
  - DVE: the serial LIF recurrence, one fused custom-DVE op per step
        u_t = select(u_{t-1} < 0.5, TAU*u_{t-1}, 0) + x_t
    (bitwise-exact vs the jax reference) plus the first 3 spike steps per
    batch (tensor_scalar is_ge -> uint8 {0,1}). Measured: DVE ops are
    cheap (~550 ns) and hide under the DMA stream, while wide uint8 Sign
    ops on ACT are ~2 us, so a 3/5 DVE/ACT spike split balances the
    engines (measured sweep: g8=2 -> 27.6 us, g8=3 -> 18.5 us w/o stores).
  - ACT: remaining spike steps via Sign(u - 0.5) -> uint8 {255,0,1}, two
    steps fused per instruction where possible (the paired membrane tiles
    are column-adjacent so one Sign covers [128, 2048]).
  - spikes accumulate per batch in a [128, 8*1024] uint8 tile; ONE 1 MiB
    store per batch on the ACT HWDGE ring (Pool SWDGE issue measured
    ~2 us/job of pipeline stall; SP would FIFO-block the load stream).
  - host maps uint8 (y == 1) -> fp32: is_ge gives {0,1}, Sign gives
    {255,0,1}; spike == 1 under both, so no correction pass is needed.
All arithmetic is fp32 and rounds identically to the jax reference, so the
spike train matches bitwise.

Variant "fused" is the previous baseline (all spikes on ACT, one 4 MiB
store at the end, unchunked loads).
"""

import os
import numpy as np

B, T, C, H, W = 32, 8, 128, 32, 32
HW = H * W
N_CORES = 8
BPC = B // N_CORES  # batches per core
TAU = 0.25
THRESH = 0.5

_nc_cache = {}
LAST_RESULTS = None
_LIF_OP = None


def _register_lif_op():
    """Register the fused LIF-step custom DVE op with concourse's runtime
    table (the documented extension point is appending to dve_ops.OPS).

        out = select(in0 < s0, in0 * s1, 0) + in1
            = TAU*u*[u < THRESH] + x      (s0=THRESH, s1=TAU)

    One DVE instruction per time step instead of two scalar_tensor_tensor
    passes; exact fp32 (mult by 2^-2 exact, one rounding add)."""
    global _LIF_OP
    if _LIF_OP is not None:
        return _LIF_OP
    from concourse import dve_ops
    from concourse.dve_spec import (
        Spec,
        Src0,
        Src1,
        C0,
        C1,
        Zero,
        select,
        lower,
        _has_src1,
    )
    from concourse.dve_uop import DveOpSpec

    name = "LIF_STEP_ANT"
    for op in dve_ops.OPS:
        if op.name == name:
            _LIF_OP = op
            return op

    body = select(Src0 < C0, Src0 * C1, Zero) + Src1

    def _ref(in0, in1, s0, s1, imm2):
        return (
            np.where(in0 < s0, in0.astype(np.float32) * s1, 0.0).astype(np.float32)
            + in1
        )

    spec = Spec(body=body, reference=_ref)
    opcode = dve_ops._CUSTOM_DVE_ROW_BASE + len(dve_ops.OPS)
    assert opcode < 0x20
    shas = {}
    for ver in ("v3", "v4"):
        try:
            uops = lower(spec, ver=ver)
        except Exception:
            continue
        shas[ver] = DveOpSpec(
            name=name, opcode=opcode, uops=uops, rd1_en=_has_src1(spec)
        ).sha(ver)
    op = dve_ops.DveOp(name, spec, subdim=False, uops_sha=shas)
    dve_ops.OPS.append(op)
    dve_ops._SUB_OPCODE_FOR_NAME[name] = opcode
    dve_ops.CUSTOM_DVE_SPECS[name] = spec
    _LIF_OP = op
    return op


def build_bass(reps=1, variant="direct"):
    """Per-core Bass program. reps>1 repeats the whole computation for
    loop-delta hardware timing (outputs are rewritten identically)."""
    import concourse.bacc as bacc
    import concourse.mybir as mybir
    from concourse.tile import TileContext

    f32 = mybir.dt.float32
    u8 = mybir.dt.uint8
    Alu = mybir.AluOpType
    AF = mybir.ActivationFunctionType
    lif_op = _register_lif_op()

    nc = bacc.Bacc("TRN2", target_bir_lowering=False)
    x = nc.dram_tensor("x", [BPC, C, T, HW], f32, kind="ExternalInput")
    y = nc.dram_tensor("y", [BPC, C, T, HW], u8, kind="ExternalOutput")

    if variant == "fused":
        with TileContext(nc) as tc:
            with (
                tc.tile_pool(name="xp", bufs=3) as xp,
                tc.tile_pool(name="up", bufs=3) as up,
                tc.tile_pool(name="yp", bufs=2) as yp,
                tc.tile_pool(name="cp", bufs=1) as cp,
            ):
                neg_thresh = cp.tile([C, 1], f32)
                nc.vector.memset(neg_thresh[:], -THRESH)
                for _rep in range(reps):
                    for b in range(BPC):
                        xb = xp.tile([C, T, HW], f32, tag="xb")
                        nc.sync.dma_start(xb[:], x[b])
                        if b == 0:
                            yg = yp.tile([C, BPC, T, HW], u8, tag="yb")
                        yb = yg[:, b]
                        u = None
                        for t in range(T):
                            xt = xb[:, t, :]
                            if t == 0:
                                u = xt
                            else:
                                un = up.tile([C, HW], f32, tag="u")
                                nc.vector._custom_dve(
                                    lif_op,
                                    out=un[:],
                                    in0=u if t == 1 else u[:],
                                    in1=xt,
                                    s0=THRESH,
                                    s1=TAU,
                                )
                                u = un
                            uin = u if t == 0 else u[:]
                            nc.scalar.activation(
                                yb[:, t, :], uin, AF.Sign, bias=neg_thresh[:]
                            )
                        if b == BPC - 1:
                            nc.scalar.dma_start(
                                y[:].rearrange("b c t w -> c b t w"), yg[:]
                            )
        nc.compile()
        return nc

    assert variant == "direct"
    g8 = int(os.environ.get("LIF_G8", "3"))  # spike steps on DVE per batch
    # ACT handles steps g8..7, two per Sign instruction where possible.
    # t=0's membrane is x_0 inside the load tile, so it can never pair.
    act_steps = list(range(g8, T))
    pairs = []
    i = 0
    while i < len(act_steps):
        n = 2 if i + 1 < len(act_steps) and act_steps[i] != 0 else 1
        pairs.append(tuple(act_steps[i : i + n]))
        i += n
    pair_of = {t: p for p in pairs for t in p}

    ybufs = int(os.environ.get("LIF_YBUFS", "2"))
    xbufs = int(os.environ.get("LIF_XBUFS", "3"))
    with TileContext(nc) as tc:
        with (
            tc.tile_pool(name="xp", bufs=xbufs) as xp,
            tc.tile_pool(name="up", bufs=3) as up,
            tc.tile_pool(name="yp", bufs=ybufs) as yp,
            tc.tile_pool(name="cp", bufs=1) as cp,
        ):
            neg_thresh = cp.tile([C, 1], f32)
            nc.vector.memset(neg_thresh[:], -THRESH)
            store_mode = os.environ.get("LIF_SMODE", "batch")
            store_ring = os.environ.get("LIF_STORE", "act")
            sdelay = int(os.environ.get("LIF_SDELAY", "2"))
            pending = []  # (issue_counter, dram_ap, sbuf_ap)
            gctr = 0
            for _rep in range(reps):
                yg = None
                for b in range(BPC):
                    xb = xp.tile([C, T, HW], f32, tag="xb")
                    for h in range(0, T, 2):
                        nc.sync.dma_start(xb[:, h : h + 2], x[b, :, h : h + 2])
                    # same-queue delayed stores: a store job issued on SP
                    # right after this batch's load jobs, for a batch whose
                    # spikes finished >= sdelay batches ago. Keeps reads and
                    # writes on ONE HWDGE queue (two concurrent queues
                    # measured 4.5x bandwidth loss) without sem-blocking
                    # the load prefetch.
                    while pending and pending[0][0] <= gctr - sdelay:
                        _, dram_ap, sb_ap = pending.pop(0)
                        nc.sync.dma_start(dram_ap, sb_ap)
                    if store_mode == "rep":
                        if b == 0:
                            yg = yp.tile([C, BPC, T, HW], u8, tag="yg")
                        sy = yg[:, b]
                    else:
                        sy = yp.tile([C, T, HW], u8, tag="sy")
                    uap = {0: xb[:, 0, :]}
                    ptile = {}
                    for t in range(T):
                        if t >= 1:
                            p = pair_of.get(t)
                            if p is not None and len(p) == 2:
                                if t == p[0]:
                                    ptile[p] = up.tile([C, 2, HW], f32, tag="u2", name="u2")
                                dst = ptile[p][:, t - p[0], :]
                            else:
                                u1 = up.tile([C, HW], f32, tag="u1", name="u1")
                                dst = u1[:]
                            nc.vector._custom_dve(
                                lif_op,
                                out=dst,
                                in0=uap[t - 1],
                                in1=xb[:, t, :],
                                s0=THRESH,
                                s1=TAU,
                            )
                            uap[t] = dst
                        if t < g8:
                            # DVE spike: {0,1} uint8
                            nc.vector.tensor_scalar(
                                sy[:, t, :], uap[t], THRESH, None, Alu.is_ge
                            )
                        else:
                            p = pair_of[t]
                            if t == p[-1]:
                                # ACT spike(s): Sign -> {255,0,1} uint8
                                if len(p) == 2:
                                    nc.scalar.activation(
                                        sy[:, p[0] : p[0] + 2, :],
                                        ptile[p][:],
                                        AF.Sign,
                                        bias=neg_thresh[:],
                                    )
                                else:
                                    nc.scalar.activation(
                                        sy[:, t, :],
                                        uap[t],
                                        AF.Sign,
                                        bias=neg_thresh[:],
                                    )
                    store_eng = {
                        "pool": nc.gpsimd,
                        "act": nc.scalar,
                        "sp": nc.sync,
                        "dve": nc.vector,
                    }.get(store_ring)
                    nostore = (
                        os.environ.get("LIF_NOSTORE", "0") == "1"
                        and _rep != reps - 1
                    )
                    if store_mode == "rep":
                        if b == BPC - 1 and not nostore:
                            store_eng.dma_start(
                                y[:].rearrange("b c t w -> c b t w"), yg[:]
                            )
                    elif store_ring == "spd":
                        if not nostore:
                            pending.append((gctr, y[b], sy[:]))
                    elif not nostore:
                        store_eng.dma_start(y[b], sy[:])
                    gctr += 1
            for _, dram_ap, sb_ap in pending:
                nc.sync.dma_start(dram_ap, sb_ap)
    nc.compile()
    return nc


def _get_nc():
    variant = os.environ.get("LIF_VARIANT", "direct")
    key = (variant, os.environ.get("LIF_G8", "3"), os.environ.get("LIF_STORE", "act"), os.environ.get("LIF_SMODE", "batch"), os.environ.get("LIF_YBUFS", "2"))
    if key not in _nc_cache:
        _nc_cache[key] = build_bass(variant=variant)
    return _nc_cache[key], variant


def kernel(x):
    global LAST_RESULTS
    from concourse import bass_utils

    x = np.asarray(x)
    assert x.shape == (B, T, C, H, W) and x.dtype == np.float32
    # shard to per-core c-major [BPC, C, T, HW] (the copy happens anyway)
    xs = np.moveaxis(x.reshape(B, T, C, HW), 1, 2)
    nc, variant = _get_nc()
    in_maps = [
        {"x": np.ascontiguousarray(xs[i * BPC : (i + 1) * BPC])}
        for i in range(N_CORES)
    ]
    res = bass_utils.run_bass_kernel_spmd(
        nc,
        in_maps,
        core_ids=list(range(N_CORES)),
        trace=bool(int(os.environ.get("LIF_TRACE", "0"))),
    )
    LAST_RESULTS = res
    out = np.empty((B, T, C, HW), dtype=np.float32)
    for i in range(N_CORES):
        # DVE is_ge gives {0,1}; ACT Sign gives {-1,0,+1} = {255,0,1} in
        # uint8. spike == 1 under both conventions.
        yi = np.moveaxis(res.results[i]["y"], 1, 2)
        out[i * BPC : (i + 1) * BPC] = yi == 1
    return out.reshape(B, T, C, H, W)


# revision 20
# speedup vs baseline: 1.2727x; 1.2727x over previous
"""LIF spike kernel for Trainium2 (Bass/Tile), data-parallel over 8 NeuronCores.

Problem: x [32, 8, 128, 32, 32] fp32 -> spikes [32, 8, 128, 32, 32] fp32
    mem_t = mem_{t-1} * 0.25 + x_t ; spike = (mem >= 0.5) ; mem *= (1 - spike)

Sharding: batch dim (32) split 4-per-core across 8 cores; no cross-core comm.

Per-core device program, variant "direct" (default), measured-driven design:
  - loads: x host-sharded to c-major [4, 128, 8, 1024]; per batch FOUR 1 MiB
    dma_start jobs ([128, 2, 1024] slices, 8 KiB per-partition descriptors).
    Measured: chunked jobs sustain ~1.1 TB/s vs ~0.6 TB/s for one 4 MiB job,
    so the 16.8 MB read floor is ~15.5 us.
  - DVE: the serial LIF recurrence, one fused custom-DVE op per step
        u_t = select(u_{t-1} < 0.5, TAU*u_{t-1}, 0) + x_t
    (bitwise-exact vs the jax reference) plus the first 3 spike steps per
    batch (tensor_scalar is_ge -> uint8 {0,1}). Measured: DVE ops are
    cheap (~550 ns) and hide under the DMA stream, while wide uint8 Sign
    ops on ACT are ~2 us, so a 3/5 DVE/ACT spike split balances the
    engines (measured sweep: g8=2 -> 27.6 us, g8=3 -> 18.5 us w/o stores).
  - ACT: remaining spike steps via Sign(u - 0.5) -> uint8 {255,0,1}, two
    steps fused per instruction where possible (the paired membrane tiles
    are column-adjacent so one Sign covers [128, 2048]).
  - spikes accumulate per batch in a [128, 8*1024] uint8 tile; ONE 1 MiB
    store per batch on the ACT HWDGE ring (Pool SWDGE issue measured
    ~2 us/job of pipeline stall; SP would FIFO-block the load stream).
  - host maps uint8 (y == 1) -> fp32: is_ge gives {0,1}, Sign gives
    {255,0,1}; spike == 1 under both, so no correction pass is needed.
All arithmetic is fp32 and rounds identically to the jax reference, so the
spike train matches bitwise.

Variant "fused" is the previous baseline (all spikes on ACT, one 4 MiB
store at the end, unchunked loads).
"""

import os
import numpy as np

B, T, C, H, W = 32, 8, 128, 32, 32
HW = H * W
N_CORES = 8
BPC = B // N_CORES  # batches per core
TAU = 0.25
THRESH = 0.5

_nc_cache = {}
LAST_RESULTS = None
_LIF_OP = None


def _register_lif_op():
    """Register the fused LIF-step custom DVE op with concourse's runtime
    table (the documented extension point is appending to dve_ops.OPS).

        out = select(in0 < s0, in0 * s1, 0) + in1
            = TAU*u*[u < THRESH] + x      (s0=THRESH, s1=TAU)

    One DVE instruction per time step instead of two scalar_tensor_tensor
    passes; exact fp32 (mult by 2^-2 exact, one rounding add)."""
    global _LIF_OP
    if _LIF_OP is not None:
        return _LIF_OP
    from concourse import dve_ops
    from concourse.dve_spec import (
        Spec,
        Src0,
        Src1,
        C0,
        C1,
        Zero,
        select,
        lower,
        _has_src1,
    )
    from concourse.dve_uop import DveOpSpec

    name = "LIF_STEP_ANT"
    for op in dve_ops.OPS:
        if op.name == name:
            _LIF_OP = op
            return op

    body = select(Src0 < C0, Src0 * C1, Zero) + Src1

    def _ref(in0, in1, s0, s1, imm2):
        return (
            np.where(in0 < s0, in0.astype(np.float32) * s1, 0.0).astype(np.float32)
            + in1
        )

    spec = Spec(body=body, reference=_ref)
    opcode = dve_ops._CUSTOM_DVE_ROW_BASE + len(dve_ops.OPS)
    assert opcode < 0x20
    shas = {}
    for ver in ("v3", "v4"):
        try:
            uops = lower(spec, ver=ver)
        except Exception:
            continue
        shas[ver] = DveOpSpec(
            name=name, opcode=opcode, uops=uops, rd1_en=_has_src1(spec)
        ).sha(ver)
    op = dve_ops.DveOp(name, spec, subdim=False, uops_sha=shas)
    dve_ops.OPS.append(op)
    dve_ops._SUB_OPCODE_FOR_NAME[name] = opcode
    dve_ops.CUSTOM_DVE_SPECS[name] = spec
    _LIF_OP = op
    return op


def build_bass(reps=1, variant="direct"):
    """Per-core Bass program. reps>1 repeats the whole computation for
    loop-delta hardware timing (outputs are rewritten identically)."""
    import concourse.bacc as bacc
    import concourse.mybir as mybir
    from concourse.tile import TileContext

    f32 = mybir.dt.float32
    u8 = mybir.dt.uint8
    Alu = mybir.AluOpType
    AF = mybir.ActivationFunctionType
    lif_op = _register_lif_op()

    nc = bacc.Bacc("TRN2", target_bir_lowering=False)
    x = nc.dram_tensor("x", [BPC, C, T, HW], f32, kind="ExternalInput")
    y = nc.dram_tensor("y", [BPC, C, T, HW], u8, kind="ExternalOutput")

    if variant == "fused":
        with TileContext(nc) as tc:
            with (
                tc.tile_pool(name="xp", bufs=3) as xp,
                tc.tile_pool(name="up", bufs=3) as up,
                tc.tile_pool(name="yp", bufs=2) as yp,
                tc.tile_pool(name="cp", bufs=1) as cp,
            ):
                neg_thresh = cp.tile([C, 1], f32)
                nc.vector.memset(neg_thresh[:], -THRESH)
                for _rep in range(reps):
                    for b in range(BPC):
                        xb = xp.tile([C, T, HW], f32, tag="xb")
                        nc.sync.dma_start(xb[:], x[b])
                        if b == 0:
                            yg = yp.tile([C, BPC, T, HW], u8, tag="yb")
                        yb = yg[:, b]
                        u = None
                        for t in range(T):
                            xt = xb[:, t, :]
                            if t == 0:
                                u = xt
                            else:
                                un = up.tile([C, HW], f32, tag="u")
                                nc.vector._custom_dve(
                                    lif_op,
                                    out=un[:],
                                    in0=u if t == 1 else u[:],
                                    in1=xt,
                                    s0=THRESH,
                                    s1=TAU,
                                )
                                u = un
                            uin = u if t == 0 else u[:]
                            nc.scalar.activation(
                                yb[:, t, :], uin, AF.Sign, bias=neg_thresh[:]
                            )
                        if b == BPC - 1:
                            nc.scalar.dma_start(
                                y[:].rearrange("b c t w -> c b t w"), yg[:]
                            )
        nc.compile()
        return nc

    assert variant == "direct"
    g8 = int(os.environ.get("LIF_G8", "3"))  # spike steps on DVE per batch
    # ACT handles steps g8..7, two per Sign instruction where possible.
    # t=0's membrane is x_0 inside the load tile, so it can never pair.
    act_steps = list(range(g8, T))
    pairs = []
    i = 0
    while i < len(act_steps):
        n = 2 if i + 1 < len(act_steps) and act_steps[i] != 0 else 1
        pairs.append(tuple(act_steps[i : i + n]))
        i += n
    pair_of = {t: p for p in pairs for t in p}

    ybufs = int(os.environ.get("LIF_YBUFS", "2"))
    xbufs = int(os.environ.get("LIF_XBUFS", "3"))
    with TileContext(nc) as tc:
        with (
            tc.tile_pool(name="xp", bufs=xbufs) as xp,
            tc.tile_pool(name="up", bufs=3) as up,
            tc.tile_pool(name="yp", bufs=ybufs) as yp,
            tc.tile_pool(name="cp", bufs=1) as cp,
        ):
            neg_thresh = cp.tile([C, 1], f32)
            nc.vector.memset(neg_thresh[:], -THRESH)
            store_mode = os.environ.get("LIF_SMODE", "batch")
            store_ring = os.environ.get("LIF_STORE", "act")
            sdelay = int(os.environ.get("LIF_SDELAY", "2"))
            pending = []  # (issue_counter, dram_ap, sbuf_ap)
            gctr = 0
            for _rep in range(reps):
                yg = None
                for b in range(BPC):
                    xb = xp.tile([C, T, HW], f32, tag="xb")
                    for h in range(0, T, 2):
                        nc.sync.dma_start(xb[:, h : h + 2], x[b, :, h : h + 2])
                    # same-queue delayed stores: a store job issued on SP
                    # right after this batch's load jobs, for a batch whose
                    # spikes finished >= sdelay batches ago. Keeps reads and
                    # writes on ONE HWDGE queue (two concurrent queues
                    # measured 4.5x bandwidth loss) without sem-blocking
                    # the load prefetch.
                    while pending and pending[0][0] <= gctr - sdelay:
                        _, dram_ap, sb_ap = pending.pop(0)
                        nc.sync.dma_start(dram_ap, sb_ap)
                    if store_mode == "rep":
                        if b == 0:
                            yg = yp.tile([C, BPC, T, HW], u8, tag="yg")
                        sy = yg[:, b]
                    else:
                        sy = yp.tile([C, T, HW], u8, tag="sy")
                    uap = {0: xb[:, 0, :]}
                    ptile = {}
                    for t in range(T):
                        if t >= 1:
                            p = pair_of.get(t)
                            if p is not None and len(p) == 2:
                                if t == p[0]:
                                    ptile[p] = up.tile([C, 2, HW], f32, tag="u2", name="u2")
                                dst = ptile[p][:, t - p[0], :]
                            else:
                                u1 = up.tile([C, HW], f32, tag="u1", name="u1")
                                dst = u1[:]
                            nc.vector._custom_dve(
                                lif_op,
                                out=dst,
                                in0=uap[t - 1],
                                in1=xb[:, t, :],
                                s0=THRESH,
                                s1=TAU,
                            )
                            uap[t] = dst
                        if t < g8:
                            # DVE spike: {0,1} uint8
                            nc.vector.tensor_scalar(
                                sy[:, t, :], uap[t], THRESH, None, Alu.is_ge
                            )
                        else:
                            p = pair_of[t]
                            if t == p[-1]:
                                # ACT spike(s): Sign -> {255,0,1} uint8
                                if len(p) == 2:
                                    nc.scalar.activation(
                                        sy[:, p[0] : p[0] + 2, :],
                                        ptile[p][:],
                                        AF.Sign,
                                        bias=neg_thresh[:],
                                    )
                                else:
                                    nc.scalar.activation(
                                        sy[:, t, :],
                                        uap[t],
                                        AF.Sign,
                                        bias=neg_thresh[:],
                                    )
                    store_eng = {
                        "pool": nc.gpsimd,
                        "act": nc.scalar,
                        "sp": nc.sync,
                        "dve": nc.vector,
                    }.get(store_ring)
                    nostore = (
                        os.environ.get("LIF_NOSTORE", "0") == "1"
                        and _rep != reps - 1
                    )
                    if store_mode == "rep":
                        if b == BPC - 1 and not nostore:
                            dram = y[:].rearrange("b c t w -> c b t w")
                            if store_ring == "spd":
                                pending.append((gctr, dram, yg[:]))
                            else:
                                store_eng.dma_start(dram, yg[:])
                    elif store_ring == "spd":
                        if not nostore:
                            pending.append((gctr, y[b], sy[:]))
                    elif not nostore:
                        store_eng.dma_start(y[b], sy[:])
                    gctr += 1
            for _, dram_ap, sb_ap in pending:
                nc.sync.dma_start(dram_ap, sb_ap)
    nc.compile()
    return nc


def _get_nc():
    variant = os.environ.get("LIF_VARIANT", "direct")
    key = (variant, os.environ.get("LIF_G8", "3"), os.environ.get("LIF_STORE", "act"), os.environ.get("LIF_SMODE", "batch"), os.environ.get("LIF_YBUFS", "2"))
    if key not in _nc_cache:
        _nc_cache[key] = build_bass(variant=variant)
    return _nc_cache[key], variant


def kernel(x):
    global LAST_RESULTS
    from concourse import bass_utils

    x = np.asarray(x)
    assert x.shape == (B, T, C, H, W) and x.dtype == np.float32
    # shard to per-core c-major [BPC, C, T, HW] (the copy happens anyway)
    xs = np.moveaxis(x.reshape(B, T, C, HW), 1, 2)
    nc, variant = _get_nc()
    in_maps = [
        {"x": np.ascontiguousarray(xs[i * BPC : (i + 1) * BPC])}
        for i in range(N_CORES)
    ]
    res = bass_utils.run_bass_kernel_spmd(
        nc,
        in_maps,
        core_ids=list(range(N_CORES)),
        trace=bool(int(os.environ.get("LIF_TRACE", "0"))),
    )
    LAST_RESULTS = res
    out = np.empty((B, T, C, HW), dtype=np.float32)
    for i in range(N_CORES):
        # DVE is_ge gives {0,1}; ACT Sign gives {-1,0,+1} = {255,0,1} in
        # uint8. spike == 1 under both conventions.
        yi = np.moveaxis(res.results[i]["y"], 1, 2)
        out[i * BPC : (i + 1) * BPC] = yi == 1
    return out.reshape(B, T, C, H, W)
